# revision 5
# baseline (speedup 1.0000x reference)
"""ArcFace (AngularPenaltySMLoss) distributed Bass kernel for 8 TRN2 NeuronCores.

Strategy (vocab/tensor parallel, per sharding hint):
  - W [50000, 512] is sharded along the class dim: core k owns classes
    [6250k, 6250(k+1)), padded to 6272 = 49*128 columns (zero rows -> logit 0
    -> exp contributes exactly 1.0 per pad, subtracted as a constant on the
    host when combining).
  - Host packs the shard TRANSPOSED + bf16: wt [512, 6272] so the moving
    matmul operand needs no on-device transpose. x is shipped both raw f32
    [4096, 512] (for row norms) and transposed bf16 xt [512, 4096].
  - Device (per core): logits tile [128 rows, c-chunk] = xT.T @ wT on
    TensorE (bf16 inputs, f32 PSUM accumulate over 4 K-chunks). ScalarE
    applies exp(S * logit / ||x_n||) using the per-partition scale AP, and
    its fused accum_out emits the row-wise exp-sums directly - the
    [4096 x 6250] exp'd logits never leave the chip.
  - Target path (row parallel): core k owns rows [512k, 512(k+1)); it
    indirect-DMA-gathers W[target_i] rows from a full f32 copy of W and
    dots them against raw f32 x rows on VectorE (tensor_tensor_reduce).
  - Each core outputs [128, 36] partials: 32 columns of per-row local
    exp-sums (row n = 128*col + p) + 4 columns of target-cos for its rows.
    The host sums the 8 partial buffers (the all-reduce) and finishes the
    O(N) scalar tail:
      num = S*(t*cos(m) - sqrt(1-t^2)*sin(m))     (== S*cos(acos(t)+m))
      L   = num - log(exp(num) + full_sum - pads - exp(S*t))
      out = -mean(L)
"""

import functools
import math
import sys

import numpy as np

sys.path.insert(0, "/opt/trn_rl_repo")

N, D, C = 4096, 512, 50000
NCORES = 8
CSH = C // NCORES          # 6250 classes per core
CPAD = 6272                # 49*128 = 12.25 * 512
S = 30.0
MARG = 0.4
EPS = 1e-7
PADS_TOTAL = float((CPAD - CSH) * NCORES)   # 176 pad classes, each exp(0)=1
ROWS_PER_CORE = N // NCORES                 # 512
NTILES = N // 128                           # 32
KT = D // 128                               # 4
# c-groups per row-tile: (offset, width, n 512-chunks); 6272 = 3*2048 + 128
GROUPS = [(0, 2048, 4), (2048, 2048, 4), (4096, 2048, 4), (6144, 128, 1)]
NG = len(GROUPS)


def build_graph():
    from concourse import bacc, bass, mybir, tile

    f32 = mybir.dt.float32
    bf16 = mybir.dt.bfloat16
    i32 = mybir.dt.int32
    AF = mybir.ActivationFunctionType
    ALU = mybir.AluOpType

    nc = bacc.Bacc(
        "TRN2",
        target_bir_lowering=False,
        debug=False,
        enable_asserts=False,
        num_devices=NCORES,
    )

    x_d = nc.dram_tensor("x", [N, D], f32, kind="ExternalInput")
    xt_d = nc.dram_tensor("xt", [D, N], bf16, kind="ExternalInput")
    wt_d = nc.dram_tensor("wt", [D, CPAD], bf16, kind="ExternalInput")
    wf_d = nc.dram_tensor("wfull", [C, D], f32, kind="ExternalInput")
    xo_d = nc.dram_tensor("xown", [ROWS_PER_CORE, D], f32, kind="ExternalInput")
    ti_d = nc.dram_tensor("tgti", [128, 4], i32, kind="ExternalInput")
    out_d = nc.dram_tensor("out", [128, 36], f32, kind="ExternalOutput")

    with tile.TileContext(nc) as tc:
        with (
            tc.tile_pool(name="big", bufs=1) as bigp,
            tc.tile_pool(name="xs", bufs=4) as xsp,
            tc.tile_pool(name="wk", bufs=2) as wk,
            tc.tile_pool(name="ps", bufs=2, space="PSUM") as pp,
        ):
            # ---------- persistent operands ----------
            wt_sb = []
            xt_sb = []
            for k in range(KT):
                w_t = bigp.tile([128, CPAD], bf16, name=f"wtsb{k}", tag=f"wtsb{k}")
                nc.sync.dma_start(w_t[:], wt_d.ap()[k * 128:(k + 1) * 128, :])
                wt_sb.append(w_t)
                x_t = bigp.tile([128, N], bf16, name=f"xtsb{k}", tag=f"xtsb{k}")
                nc.sync.dma_start(x_t[:], xt_d.ap()[k * 128:(k + 1) * 128, :])
                xt_sb.append(x_t)

            tgti_sb = bigp.tile([128, 4], i32, name="tgti_sb")
            nc.sync.dma_start(tgti_sb[:], ti_d.ap()[:, :])

            SS32 = bigp.tile([128, NTILES], f32, name="SS32")     # sum(x^2) per row
            NRM = bigp.tile([128, NTILES], f32, name="NRM")
            RNS = bigp.tile([128, NTILES], f32, name="RNS")       # S / ||x_n||
            SSG = bigp.tile([128, NTILES * NG], f32, name="SSG")  # per-group expsums
            CONTRIB = bigp.tile([128, 36], f32, name="CONTRIB")

            # ---------- row norms for all 4096 rows ----------
            for j in range(NTILES):
                xtile = xsp.tile([128, D], f32, name="xtile", tag="xtile")
                nc.sync.dma_start(xtile[:], x_d.ap()[j * 128:(j + 1) * 128, :])
                # NB: tensor_tensor_reduce hard-crashes the device on this
                # runtime (NRT_EXEC_UNIT_UNRECOVERABLE) - use mul + reduce.
                tsq = wk.tile([128, D], f32, name="tsq", tag="tsq")
                nc.vector.tensor_mul(tsq[:], xtile[:], xtile[:])
                nc.vector.tensor_reduce(
                    SS32[:, j:j + 1], tsq[:], mybir.AxisListType.X, ALU.add
                )
            # sqrt(ss)/S then reciprocal -> S/||x||  (Rsqrt ACT is banned)
            nc.scalar.activation(NRM[:], SS32[:], AF.Sqrt, scale=1.0 / (S * S))
            nc.vector.reciprocal(RNS[:], NRM[:])

            # ---------- own-row norms (for the target path) ----------
            xo_sb = []
            SSO = bigp.tile([128, 4], f32, name="SSO")
            NRO = bigp.tile([128, 4], f32, name="NRO")
            RNO = bigp.tile([128, 4], f32, name="RNO")
            for j in range(4):
                xo_t = bigp.tile([128, D], f32, name=f"xo{j}", tag=f"xo{j}")
                nc.sync.dma_start(xo_t[:], xo_d.ap()[j * 128:(j + 1) * 128, :])
                xo_sb.append(xo_t)
                osq = wk.tile([128, D], f32, name="osq", tag="tsq")
                nc.vector.tensor_mul(osq[:], xo_t[:], xo_t[:])
                nc.vector.tensor_reduce(
                    SSO[:, j:j + 1], osq[:], mybir.AxisListType.X, ALU.add
                )
            nc.scalar.activation(NRO[:], SSO[:], AF.Sqrt)
            nc.vector.reciprocal(RNO[:], NRO[:])

            # ---------- target gather + dot (f32 exact) ----------
            TGD = bigp.tile([128, 4], f32, name="TGD")
            for j in range(4):
                wrow = wk.tile([128, D], f32, name="wrow", tag="wrow")
                nc.gpsimd.indirect_dma_start(
                    out=wrow[:],
                    out_offset=None,
                    in_=wf_d.ap(),
                    in_offset=bass.IndirectOffsetOnAxis(
                        ap=tgti_sb[:, j:j + 1], axis=0
                    ),
                )
                gsq = wk.tile([128, D], f32, name="gsq", tag="tsq")
                nc.vector.tensor_mul(gsq[:], wrow[:], xo_sb[j][:])
                nc.vector.tensor_reduce(
                    TGD[:, j:j + 1], gsq[:], mybir.AxisListType.X, ALU.add
                )
            nc.vector.tensor_mul(CONTRIB[:, 32:36], TGD[:], RNO[:])

            # ---------- main matmul + fused exp/row-sum ----------
            for j in range(NTILES):
                for g, (c0, width, nch) in enumerate(GROUPS):
                    pg = pp.tile([128, 2048], f32, name="pg", tag="pg")
                    for cc in range(nch):
                        ncol = min(512, width - cc * 512)
                        for k in range(KT):
                            nc.tensor.matmul(
                                out=pg[:, cc * 512:cc * 512 + ncol],
                                lhsT=xt_sb[k][:, j * 128:(j + 1) * 128],
                                rhs=wt_sb[k][:, c0 + cc * 512:c0 + cc * 512 + ncol],
                                start=(k == 0),
                                stop=(k == KT - 1),
                            )
                    esink = wk.tile([128, 2048], bf16, name="esink", tag="esink")
                    nc.scalar.activation(
                        out=esink[:, :width],
                        in_=pg[:, :width],
                        func=AF.Exp,
                        scale=RNS[:, j:j + 1],
                        accum_out=SSG[:, j * NG + g:j * NG + g + 1],
                    )

            # row-wise full exp sums: reduce the NG group-sums per row-tile
            nc.vector.tensor_reduce(
                CONTRIB[:, 0:32],
                SSG[:].rearrange("p (a b) -> p a b", b=NG),
                mybir.AxisListType.X,
                ALU.add,
            )

            nc.sync.dma_start(out_d.ap()[:, :], CONTRIB[:])

    nc.compile()
    return nc


@functools.lru_cache(maxsize=1)
def _compiled():
    return build_graph()


def _prep_in_maps(x, W, target):
    import ml_dtypes

    bf16 = ml_dtypes.bfloat16
    x = np.asarray(x, dtype=np.float32)
    W = np.asarray(W, dtype=np.float32)
    target = np.asarray(target, dtype=np.int32)

    xt = np.ascontiguousarray(x.T).astype(bf16)
    in_maps = []
    for k in range(NCORES):
        wt = np.zeros((D, CPAD), dtype=bf16)
        wt[:, :CSH] = np.ascontiguousarray(W[k * CSH:(k + 1) * CSH].T).astype(bf16)
        town = target[k * ROWS_PER_CORE:(k + 1) * ROWS_PER_CORE]
        tgti = np.ascontiguousarray(town.reshape(4, 128).T).astype(np.int32)
        in_maps.append(
            {
                "x": x,
                "xt": xt,
                "wt": wt,
                "wfull": W,
                "xown": np.ascontiguousarray(
                    x[k * ROWS_PER_CORE:(k + 1) * ROWS_PER_CORE]
                ),
                "tgti": tgti,
            }
        )
    return in_maps


def _combine(parts):
    """Host-side all-reduce of the per-core [128, 36] partials + scalar tail."""
    fs = np.zeros((128, 32), dtype=np.float64)
    tg = np.zeros(N, dtype=np.float64)
    for k, p in enumerate(parts):
        p = np.asarray(p, dtype=np.float64)
        fs += p[:, 0:32]
        # core k's target-cos for rows [512k, 512(k+1)): col j <-> n = 512k+128j+p
        tg[ROWS_PER_CORE * k:ROWS_PER_CORE * (k + 1)] = p[:, 32:36].T.reshape(-1)
    # fs[p, col] <-> row n = 128*col + p
    full_sum = fs.T.reshape(-1)  # [4096]
    tcl = np.clip(tg, -1.0 + EPS, 1.0 - EPS)
    num = S * (tcl * math.cos(MARG) - np.sqrt(1.0 - tcl * tcl) * math.sin(MARG))
    excl = full_sum - PADS_TOTAL - np.exp(S * tg)
    denom = np.exp(num) + excl
    L = num - np.log(denom)
    return np.float32(-np.mean(L))


def kernel_run(x, W, target, trace=False, **kw):
    """Returns (loss_scalar, BassKernelResults)."""
    from concourse import bass_utils

    nc = _compiled()
    in_maps = _prep_in_maps(x, W, target)
    res = bass_utils.run_bass_kernel_spmd(
        nc, in_maps, core_ids=list(range(NCORES)), trace=trace, **kw
    )
    loss = _combine([r["out"] for r in res.results])
    return np.asarray(loss, dtype=np.float32), res


def kernel(x, W, target):
    loss, _ = kernel_run(x, W, target, trace=False)
    return loss


if __name__ == "__main__":
    nc = build_graph()
    print("graph built + compiled OK")


# revision 9
# speedup vs baseline: 1.5445x; 1.5445x over previous
"""ArcFace (AngularPenaltySMLoss) distributed Bass kernel for 8 TRN2 NeuronCores.

Strategy (vocab/tensor parallel, per sharding hint):
  - W [50000, 512] is sharded along the class dim: core k owns classes
    [6250k, 6250(k+1)), padded to 6272 = 49*128 columns (zero rows -> logit 0
    -> exp contributes exactly 1.0 per pad, subtracted as a constant on the
    host when combining).
  - Host packs the shard TRANSPOSED + bf16: wt [512, 6272] so the moving
    matmul operand needs no on-device transpose. x is shipped both raw f32
    [4096, 512] (for row norms) and transposed bf16 xt [512, 4096].
  - Device (per core): logits tile [128 rows, c-chunk] = xT.T @ wT on
    TensorE (bf16 inputs, f32 PSUM accumulate over 4 K-chunks). ScalarE
    applies exp(S * logit / ||x_n||) using the per-partition scale AP, and
    its fused accum_out emits the row-wise exp-sums directly - the
    [4096 x 6250] exp'd logits never leave the chip.
  - Target path (row parallel): core k owns rows [512k, 512(k+1)); it
    indirect-DMA-gathers W[target_i] rows from a full f32 copy of W and
    dots them against raw f32 x rows on VectorE (tensor_tensor_reduce).
  - Each core outputs [128, 36] partials: 32 columns of per-row local
    exp-sums (row n = 128*col + p) + 4 columns of target-cos for its rows.
    The host sums the 8 partial buffers (the all-reduce) and finishes the
    O(N) scalar tail:
      num = S*(t*cos(m) - sqrt(1-t^2)*sin(m))     (== S*cos(acos(t)+m))
      L   = num - log(exp(num) + full_sum - pads - exp(S*t))
      out = -mean(L)
"""

import functools
import math
import sys

import numpy as np

sys.path.insert(0, "/opt/trn_rl_repo")

N, D, C = 4096, 512, 50000
NCORES = 8
CSH = C // NCORES          # 6250 classes per core
CPAD = 6272                # 49*128 = 12.25 * 512
S = 30.0
MARG = 0.4
EPS = 1e-7
PADS_TOTAL = float((CPAD - CSH) * NCORES)   # 176 pad classes, each exp(0)=1
ROWS_PER_CORE = N // NCORES                 # 512
NTILES = N // 128                           # 32
KT = D // 128                               # 4
# c-groups per row-tile: (offset, width, n 512-chunks); 6272 = 6*1024 + 128.
# 1024-wide groups (2 PSUM banks) x 4 pool slots give ScalarE ~3 group-times
# of slack to drain each group, so the PE never waits on the Exp pass.
GROUPS = [(g * 1024, 1024, 2) for g in range(6)] + [(6144, 128, 1)]
NG = len(GROUPS)


def build_graph():
    from concourse import bacc, bass, mybir, tile

    f32 = mybir.dt.float32
    bf16 = mybir.dt.bfloat16
    i32 = mybir.dt.int32
    AF = mybir.ActivationFunctionType
    ALU = mybir.AluOpType

    nc = bacc.Bacc(
        "TRN2",
        target_bir_lowering=False,
        debug=False,
        enable_asserts=False,
        num_devices=NCORES,
    )

    x_d = nc.dram_tensor("x", [N, D], f32, kind="ExternalInput")
    xt_d = nc.dram_tensor("xt", [D, N], bf16, kind="ExternalInput")
    wt_d = nc.dram_tensor("wt", [D, CPAD], bf16, kind="ExternalInput")
    wf_d = nc.dram_tensor("wfull", [C, D], f32, kind="ExternalInput")
    xo_d = nc.dram_tensor("xown", [ROWS_PER_CORE, D], f32, kind="ExternalInput")
    ti_d = nc.dram_tensor("tgti", [128, 4], i32, kind="ExternalInput")
    out_d = nc.dram_tensor("out", [128, 36], f32, kind="ExternalOutput")

    with tile.TileContext(nc) as tc:
        with (
            tc.tile_pool(name="big", bufs=1) as bigp,
            tc.tile_pool(name="xs", bufs=4) as xsp,
            tc.tile_pool(name="wk", bufs=2) as wk,
            tc.tile_pool(name="ps", bufs=4, space="PSUM") as pp,
        ):
            # ---------- persistent operands ----------
            wt_sb = []
            xt_sb = []
            for k in range(KT):
                w_t = bigp.tile([128, CPAD], bf16, name=f"wtsb{k}", tag=f"wtsb{k}")
                nc.sync.dma_start(w_t[:], wt_d.ap()[k * 128:(k + 1) * 128, :])
                wt_sb.append(w_t)
                x_t = bigp.tile([128, N], bf16, name=f"xtsb{k}", tag=f"xtsb{k}")
                nc.sync.dma_start(x_t[:], xt_d.ap()[k * 128:(k + 1) * 128, :])
                xt_sb.append(x_t)

            tgti_sb = bigp.tile([128, 4], i32, name="tgti_sb")
            nc.sync.dma_start(tgti_sb[:], ti_d.ap()[:, :])

            SS32 = bigp.tile([128, NTILES], f32, name="SS32")     # sum(x^2) per row
            NRM = bigp.tile([128, NTILES], f32, name="NRM")
            RNS = bigp.tile([128, NTILES], f32, name="RNS")       # S / ||x_n||
            SSG = bigp.tile([128, NTILES * NG], f32, name="SSG")  # per-group expsums
            CONTRIB = bigp.tile([128, 36], f32, name="CONTRIB")

            # ---------- row norms for all 4096 rows ----------
            for j in range(NTILES):
                xtile = xsp.tile([128, D], f32, name="xtile", tag="xtile")
                nc.sync.dma_start(xtile[:], x_d.ap()[j * 128:(j + 1) * 128, :])
                # NB: tensor_tensor_reduce hard-crashes the device on this
                # runtime (NRT_EXEC_UNIT_UNRECOVERABLE) - use mul + reduce.
                tsq = wk.tile([128, D], f32, name="tsq", tag="tsq")
                nc.vector.tensor_mul(tsq[:], xtile[:], xtile[:])
                nc.vector.tensor_reduce(
                    SS32[:, j:j + 1], tsq[:], mybir.AxisListType.X, ALU.add
                )
                # finish norms in waves of 8 so early Exp groups aren't
                # gated on the full x stream (Rsqrt ACT is banned)
                if j % 8 == 7:
                    b0 = j - 7
                    nc.scalar.activation(
                        NRM[:, b0:j + 1], SS32[:, b0:j + 1], AF.Sqrt,
                        scale=1.0 / (S * S),
                    )
                    nc.vector.reciprocal(RNS[:, b0:j + 1], NRM[:, b0:j + 1])

            # ---------- own-row norms (for the target path) ----------
            xo_sb = []
            SSO = bigp.tile([128, 4], f32, name="SSO")
            NRO = bigp.tile([128, 4], f32, name="NRO")
            RNO = bigp.tile([128, 4], f32, name="RNO")
            for j in range(4):
                xo_t = bigp.tile([128, D], f32, name=f"xo{j}", tag=f"xo{j}")
                nc.sync.dma_start(xo_t[:], xo_d.ap()[j * 128:(j + 1) * 128, :])
                xo_sb.append(xo_t)
                osq = wk.tile([128, D], f32, name="osq", tag="tsq")
                nc.vector.tensor_mul(osq[:], xo_t[:], xo_t[:])
                nc.vector.tensor_reduce(
                    SSO[:, j:j + 1], osq[:], mybir.AxisListType.X, ALU.add
                )
            nc.scalar.activation(NRO[:], SSO[:], AF.Sqrt)
            nc.vector.reciprocal(RNO[:], NRO[:])

            # ---------- target gather + dot (f32 exact) ----------
            TGD = bigp.tile([128, 4], f32, name="TGD")
            for j in range(4):
                wrow = wk.tile([128, D], f32, name="wrow", tag="wrow")
                nc.gpsimd.indirect_dma_start(
                    out=wrow[:],
                    out_offset=None,
                    in_=wf_d.ap(),
                    in_offset=bass.IndirectOffsetOnAxis(
                        ap=tgti_sb[:, j:j + 1], axis=0
                    ),
                )
                gsq = wk.tile([128, D], f32, name="gsq", tag="tsq")
                nc.vector.tensor_mul(gsq[:], wrow[:], xo_sb[j][:])
                nc.vector.tensor_reduce(
                    TGD[:, j:j + 1], gsq[:], mybir.AxisListType.X, ALU.add
                )
            nc.vector.tensor_mul(CONTRIB[:, 32:36], TGD[:], RNO[:])

            # ---------- main matmul + fused exp/row-sum ----------
            for j in range(NTILES):
                for g, (c0, width, nch) in enumerate(GROUPS):
                    pg = pp.tile([128, 1024], f32, name="pg", tag="pg")
                    # k outer: one LDWEIGHTS serves both 512-chunks of the
                    # group (interleaved PSUM accumulation across banks)
                    for k in range(KT):
                        for cc in range(nch):
                            ncol = min(512, width - cc * 512)
                            nc.tensor.matmul(
                                out=pg[:, cc * 512:cc * 512 + ncol],
                                lhsT=xt_sb[k][:, j * 128:(j + 1) * 128],
                                rhs=wt_sb[k][:, c0 + cc * 512:c0 + cc * 512 + ncol],
                                start=(k == 0),
                                stop=(k == KT - 1),
                            )
                    esink = wk.tile([128, 1024], bf16, name="esink", tag="esink")
                    nc.scalar.activation(
                        out=esink[:, :width],
                        in_=pg[:, :width],
                        func=AF.Exp,
                        scale=RNS[:, j:j + 1],
                        accum_out=SSG[:, j * NG + g:j * NG + g + 1],
                    )

            # row-wise full exp sums: reduce the NG group-sums per row-tile
            nc.vector.tensor_reduce(
                CONTRIB[:, 0:32],
                SSG[:].rearrange("p (a b) -> p a b", b=NG),
                mybir.AxisListType.X,
                ALU.add,
            )

            nc.sync.dma_start(out_d.ap()[:, :], CONTRIB[:])

    nc.compile()
    return nc


@functools.lru_cache(maxsize=1)
def _compiled():
    return build_graph()


def _prep_in_maps(x, W, target):
    import ml_dtypes

    bf16 = ml_dtypes.bfloat16
    x = np.asarray(x, dtype=np.float32)
    W = np.asarray(W, dtype=np.float32)
    target = np.asarray(target, dtype=np.int32)

    xt = np.ascontiguousarray(x.T).astype(bf16)
    in_maps = []
    for k in range(NCORES):
        wt = np.zeros((D, CPAD), dtype=bf16)
        wt[:, :CSH] = np.ascontiguousarray(W[k * CSH:(k + 1) * CSH].T).astype(bf16)
        town = target[k * ROWS_PER_CORE:(k + 1) * ROWS_PER_CORE]
        tgti = np.ascontiguousarray(town.reshape(4, 128).T).astype(np.int32)
        in_maps.append(
            {
                "x": x,
                "xt": xt,
                "wt": wt,
                "wfull": W,
                "xown": np.ascontiguousarray(
                    x[k * ROWS_PER_CORE:(k + 1) * ROWS_PER_CORE]
                ),
                "tgti": tgti,
            }
        )
    return in_maps


def _combine(parts):
    """Host-side all-reduce of the per-core [128, 36] partials + scalar tail."""
    fs = np.zeros((128, 32), dtype=np.float64)
    tg = np.zeros(N, dtype=np.float64)
    for k, p in enumerate(parts):
        p = np.asarray(p, dtype=np.float64)
        fs += p[:, 0:32]
        # core k's target-cos for rows [512k, 512(k+1)): col j <-> n = 512k+128j+p
        tg[ROWS_PER_CORE * k:ROWS_PER_CORE * (k + 1)] = p[:, 32:36].T.reshape(-1)
    # fs[p, col] <-> row n = 128*col + p
    full_sum = fs.T.reshape(-1)  # [4096]
    tcl = np.clip(tg, -1.0 + EPS, 1.0 - EPS)
    num = S * (tcl * math.cos(MARG) - np.sqrt(1.0 - tcl * tcl) * math.sin(MARG))
    excl = full_sum - PADS_TOTAL - np.exp(S * tg)
    denom = np.exp(num) + excl
    L = num - np.log(denom)
    return np.float32(-np.mean(L))


def kernel_run(x, W, target, trace=False, **kw):
    """Returns (loss_scalar, BassKernelResults)."""
    from concourse import bass_utils

    nc = _compiled()
    in_maps = _prep_in_maps(x, W, target)
    res = bass_utils.run_bass_kernel_spmd(
        nc, in_maps, core_ids=list(range(NCORES)), trace=trace, **kw
    )
    loss = _combine([r["out"] for r in res.results])
    return np.asarray(loss, dtype=np.float32), res


def kernel(x, W, target):
    loss, _ = kernel_run(x, W, target, trace=False)
    return loss


if __name__ == "__main__":
    nc = build_graph()
    print("graph built + compiled OK")
